# revision 36
# baseline (speedup 1.0000x reference)
"""Causal self-attention (RoPE + QK-RMSNorm) Trainium2 kernel, 8-core tensor-parallel.

Problem: B=4, S=2048, E=2048, H=16 heads, D=128, fp32 in/out.
Sharding: tensor-parallel over heads -- each core computes 2 heads end-to-end
(QKV projection, RoPE, QK-norm, causal attention, output projection) and
returns a partial output [B*S, E] in bf16; the host sums the 8 partials in
fp32.

All matmul operands are bf16 (same 1 cycle/row PE rate as f32r, half the
SBUF/DMA): end-to-end numeric error ~5e-3 vs the 2e-2 gate (validated in
numpy with ml_dtypes).

Schedule (the point of this rewrite -- the PE has a 4-deep wait queue and
executes in order, so any matmul emitted while its producer is still running
parks and then BLOCKS later, ready matmuls):
  Phase A (QKV+rope+norm), per 512-column chunk c:
    - 16 e-rounds of QKV matmuls; V is produced in natural [s,d] layout
      directly via matmul(stationary=xt_tile, moving=wv) -- no PE transposes.
    - chunk c-1's rope matmuls (sum-sq ones-matmul, J rotation matmul) are
      interleaved between chunk c's e-rounds (their ACT/DVE inputs completed
      during the previous chunk) and drained to SBUF immediately by DVE.
    - chunk c-1's elementwise rope tail runs on ACT/DVE/Pool underneath
      chunk c's matmuls.
  Phase B (attention), per (q-block, head): scores into a double-buffered
  PSUM tile; exp on ACT; the ctx/rowsum matmuls are emitted one k-group
  BEHIND the scores so they never park on the ACT exp. The softmax
  reciprocal is a fast DVE Newton iteration (no ACT round trip).
  Phase C (out-proj) interleaves per q-block as before, in 512-col chunks.

PSUM = 4 mega-tiles x 2 banks, manually sliced per phase:
  T0/T1 [128,2,512]: A: q|k psums per head; B: scores double-buffer.
  T2 [128,2,4,128]:  A: V-natural psum (h,st,d); B: ctx | rowsum.
  T3 [128,2,512]:    A: rope scratch x2;        B/C: proj out double-buffer.
"""

import sys

sys.path.insert(0, "/opt/trn_rl_repo")

import numpy as np
import ml_dtypes
from contextlib import ExitStack

import concourse.bass as bass
import concourse.mybir as mybir
import concourse.tile as tile
from concourse import bacc
from concourse.bass_utils import run_bass_kernel_spmd

# Keep every ACT function this kernel uses (Exp/Ln/Square/Copy) resolvable
# only via the combined natural_log_exp_and_others table set; otherwise the
# table-load inserter alternates between exp_and_others and natural_log and
# pays a ~2.7us ACT_TABLE_LOAD on nearly every activation (~500us/run).
_orig_get_act_tables = bacc.get_activation_tables
_COMBINED = "natural_log_exp_and_others"
_KEEP = {mybir.ActivationFunctionType.Exp, mybir.ActivationFunctionType.Ln,
         mybir.ActivationFunctionType.Square, mybir.ActivationFunctionType.Copy}


def _patched_get_act_tables(arch):
    tables = _orig_get_act_tables(arch)
    if _COMBINED in tables and _KEEP <= tables[_COMBINED]:
        for name, funcs in tables.items():
            if name != _COMBINED:
                tables[name] = funcs - _KEEP
    return tables


bacc.get_activation_tables = _patched_get_act_tables

F32 = mybir.dt.float32
F32R = mybir.dt.float32r
BF16 = mybir.dt.bfloat16
AF = mybir.ActivationFunctionType
NPBF16 = ml_dtypes.bfloat16

N_CORES = 8
N_HEAD = 16
ROPE_BASE = 10000.0
QK_NORM_EPS = 1e-5

B, S, E = 4, 2048, 2048
D = E // N_HEAD          # 128
HPC = N_HEAD // N_CORES  # heads per core


def build_kernel(b_=B, s_=S, repeat=1, debug=False):
    """Build the per-core Bass program for batch size b_ and seqlen s_."""
    ROWS = b_ * s_
    QB = min(512, s_)     # q-block width in attention
    NQB = s_ // QB
    KPQ = QB // 128       # k-tiles spanned by one q-block (diag band width)
    NE = E // 128
    RC = min(512, s_)     # phase-A column chunk
    NRC = s_ // RC        # chunks per batch
    NCH = b_ * NRC        # chunks per repeat iteration

    nc = bacc.Bacc("TRN2", target_bir_lowering=False, debug=False)

    xT = nc.dram_tensor("xT", [E, ROWS], BF16, kind="ExternalInput").ap()
    wq = nc.dram_tensor("wq", [E, HPC * D], BF16, kind="ExternalInput").ap()
    wk = nc.dram_tensor("wk", [E, HPC * D], BF16, kind="ExternalInput").ap()
    wv = nc.dram_tensor("wv", [E, HPC * D], BF16, kind="ExternalInput").ap()
    wp = nc.dram_tensor("wp", [HPC * D, E], BF16, kind="ExternalInput").ap()
    cos2 = nc.dram_tensor("cos2", [128, s_], F32, kind="ExternalInput").ap()
    sin2 = nc.dram_tensor("sin2", [128, s_], F32, kind="ExternalInput").ap()
    jmat = nc.dram_tensor("jmat", [128, 128], F32R, kind="ExternalInput").ap()
    trimask = nc.dram_tensor("trimask", [128, 128], BF16, kind="ExternalInput").ap()
    onesd = nc.dram_tensor("onesd", [128, 128], F32R, kind="ExternalInput").ap()
    onesb = nc.dram_tensor("onesb", [128, 128], BF16, kind="ExternalInput").ap()
    out = nc.dram_tensor("out", [ROWS, E], BF16, kind="ExternalOutput").ap()
    if debug:
        dbg_qtn = nc.dram_tensor("dbg_qtn", [HPC, 128, s_], BF16, kind="ExternalOutput").ap()
        dbg_ktn = nc.dram_tensor("dbg_ktn", [HPC, 128, s_], BF16, kind="ExternalOutput").ap()
        dbg_vsb = nc.dram_tensor("dbg_vsb", [HPC, 128, s_ // 128, D], BF16, kind="ExternalOutput").ap()
        dbg_ctx = nc.dram_tensor("dbg_ctx", [HPC, 128, s_], BF16, kind="ExternalOutput").ap()

    LN_SCALE = 1.0 / D
    LN_BIAS = QK_NORM_EPS
    EXP_SCALE = -0.5
    EXP_BIAS = -0.25 * float(np.log(D))  # folds 1/sqrt(D) into the q,k scales

    with tile.TileContext(nc) as tc, ExitStack() as ctx:
        wpool = ctx.enter_context(tc.tile_pool(name="weights", bufs=1))
        const = ctx.enter_context(tc.tile_pool(name="const", bufs=1))
        xtp = ctx.enter_context(tc.tile_pool(name="xt", bufs=4))
        qkv = ctx.enter_context(tc.tile_pool(name="qkv", bufs=1))
        tmp = ctx.enter_context(tc.tile_pool(name="tmp", bufs=2))
        expp = ctx.enter_context(tc.tile_pool(name="expp", bufs=2))
        ctxp = ctx.enter_context(tc.tile_pool(name="ctxp", bufs=1))
        outp = ctx.enter_context(tc.tile_pool(name="outp", bufs=4))
        psp = ctx.enter_context(tc.tile_pool(name="ps", bufs=1, space="PSUM"))

        # resident weights: [128, NE, HPC*D] with contraction slice e on free dim
        wq_s = wpool.tile([128, NE, HPC * D], BF16, tag="wqs")
        wk_s = wpool.tile([128, NE, HPC * D], BF16, tag="wks")
        wv_s = wpool.tile([128, NE, HPC * D], BF16, tag="wvs")
        wp_s = wpool.tile([128, HPC, E], BF16, tag="wps")
        nc.sync.dma_start(out=wq_s, in_=wq.rearrange("(ne p) m -> p ne m", p=128))
        nc.sync.dma_start(out=wk_s, in_=wk.rearrange("(ne p) m -> p ne m", p=128))
        nc.sync.dma_start(out=wv_s, in_=wv.rearrange("(ne p) m -> p ne m", p=128))
        nc.sync.dma_start(out=wp_s, in_=wp.rearrange("(h p) m -> p h m", p=128))

        cos_s = const.tile([128, s_], F32, tag="cos")
        sin_s = const.tile([128, s_], F32, tag="sin")
        j_s = const.tile([128, 128], F32R, tag="jmat")
        tri_s = const.tile([128, 128], BF16, tag="tri")
        ones_s = const.tile([128, 128], F32R, tag="ones")
        onesb_s = const.tile([128, 128], BF16, tag="onesb")
        nc.sync.dma_start(out=ones_s, in_=onesd)
        nc.sync.dma_start(out=onesb_s, in_=onesb)
        bias_ln = const.tile([128, 1], F32, tag="bias_ln")
        bias_ex = const.tile([128, 1], F32, tag="bias_ex")
        nc.vector.memset(bias_ln, LN_BIAS)
        nc.vector.memset(bias_ex, EXP_BIAS)

        nc.sync.dma_start(out=cos_s, in_=cos2)
        nc.sync.dma_start(out=sin_s, in_=sin2)
        nc.sync.dma_start(out=j_s, in_=jmat)
        nc.sync.dma_start(out=tri_s, in_=trimask)

        # PSUM mega-tiles (2 banks each, manually sliced per phase)
        T0 = psp.tile([128, 2, 512], F32, tag="T0", name="T0")
        T1 = psp.tile([128, 2, 512], F32, tag="T1", name="T1")
        T2a = psp.tile([128, 2, 2, 128], F32, tag="T2a", name="T2a")
        T2b = psp.tile([128, 2, 2, 128], F32, tag="T2b", name="T2b")
        T3a = psp.tile([128, 512], F32, tag="T3a", name="T3a")
        T3b = psp.tile([128, 512], F32, tag="T3b", name="T3b")
        p_qk = [(T0[:, 0], T0[:, 1]), (T1[:, 0], T1[:, 1])]   # (q, k) per head

        # per-batch SBUF state (bf16)
        qtn = [qkv.tile([128, s_], BF16, tag=f"qtn{h}", name=f"qtn{h}")
               for h in range(HPC)]
        ktn = [qkv.tile([128, s_], BF16, tag=f"ktn{h}", name=f"ktn{h}")
               for h in range(HPC)]
        vsb = [qkv.tile([128, s_ // 128, D], BF16, tag=f"vsb{h}", name=f"vsb{h}")
               for h in range(HPC)]
        ctxTs = [ctxp.tile([128, s_], BF16, tag=f"ctxT{h}", name=f"ctxT{h}")
                 for h in range(HPC)]

        rep_ctx = tc.For_i(0, repeat, 1) if repeat > 1 else None
        if rep_ctx is not None:
            ctx.enter_context(rep_ctx)

        # ---- phase A helpers (software-pipelined over chunks) -----------
        # Pipeline state for the previous chunk: SBUF drains of its psums.
        prev = {}

        def drain_qk(st, pair, use_act=False):
            """psum -> SBUF raw copy of one q/k psum (the only psum reader;
            sq is computed later from the SBUF copy so it doesn't hold the
            psum tile -- tile deps are whole-tile for PSUM)."""
            h, which = divmod(pair, 2)
            psrc = p_qk[h][which]
            raw = tmp.tile([128, RC], F32R, tag="raw", name="raw", bufs=8)
            if use_act:
                nc.scalar.activation(raw, psrc, AF.Copy)
            else:
                nc.vector.tensor_copy(raw, psrc)
            st["raw"][pair] = raw

        def emit_sq(st, pair):
            sq = tmp.tile([128, RC], F32R, tag="sq", name="sq", bufs=8)
            nc.scalar.activation(sq, st["raw"][pair], AF.Square)
            st["sq"][pair] = sq

        def drain_v(st):
            c = st["c"]
            rc = c % NRC
            nst = RC // 128
            hnst = nst // 2
            for h in range(HPC):
                nc.vector.tensor_copy(
                    vsb[h][:, rc * nst:rc * nst + hnst, :], T2a[:, :, h, :])
                nc.vector.tensor_copy(
                    vsb[h][:, rc * nst + hnst:(rc + 1) * nst, :], T2b[:, :, h, :])

        def emit_rope_mms(st, pair):
            """ss/jq matmuls + immediate DVE drains for one (q/k,head) chain."""
            raw, sq = st["raw"][pair], st["sq"][pair]
            sc = (T3a, T3b)[pair % 2]
            nc.tensor.matmul(sc, ones_s, sq, start=True, stop=True)
            den = tmp.tile([128, RC], F32, tag="den", name="den", bufs=4)
            nc.vector.tensor_copy(den, sc)
            sc2 = (T3a, T3b)[(pair + 1) % 2]
            nc.tensor.matmul(sc2, j_s, raw, start=True, stop=True)
            jqr = tmp.tile([128, RC], F32, tag="jqr", name="jqr", bufs=4)
            nc.vector.tensor_copy(jqr, sc2)
            st.setdefault("den", []).append(den)
            st.setdefault("jqr", []).append(jqr)

        def emit_rope_tail(st, pair):
            """Elementwise rope+norm for one chain; writes qtn/ktn chunk c."""
            c = st["c"]
            rc = c % NRC
            csl = slice(rc * RC, rc * RC + RC)
            h, which = divmod(pair, 2)
            dst = qtn[h] if which == 0 else ktn[h]
            raw = st["raw"][pair]
            den = st["den"][pair]
            jqr = st["jqr"][pair]
            lnt = tmp.tile([128, RC], F32, tag="lnt", name="lnt", bufs=2)
            nc.scalar.activation(lnt, den, AF.Ln, scale=LN_SCALE, bias=bias_ln)
            rq = tmp.tile([128, RC], F32, tag="rq", name="rq", bufs=3)
            nc.scalar.activation(rq, lnt, AF.Exp, scale=EXP_SCALE, bias=bias_ex)
            t1 = tmp.tile([128, RC], F32, tag="t1", name="t1", bufs=2)
            nc.gpsimd.tensor_mul(t1, raw, cos_s[:, csl])
            t2 = tmp.tile([128, RC], F32, tag="t2", name="t2", bufs=2)
            nc.vector.tensor_mul(t2, jqr, sin_s[:, csl])
            t3 = tmp.tile([128, RC], F32, tag="t3", name="t3", bufs=2)
            nc.gpsimd.tensor_add(t3, t1, t2)
            nc.gpsimd.tensor_mul(dst[:, csl], t3, rq)

        def emit_chunk(c, prev_st, defer=None):
            """e-rounds for chunk c with chunk c-1's rope work interleaved.

            defer: optional list of closures (previous batch's last-qb proj
            units), one emitted per e-round to overlap with this chunk."""
            b, rc = divmod(c, NRC)
            col0 = b * s_ + rc * RC
            st = {"c": c, "raw": [None] * 4, "sq": [None] * 4}
            for e in range(NE):
                if prev_st and e in (4, 6, 8, 10):
                    emit_rope_mms(prev_st, (e - 4) // 2)
                if prev_st and e in (12, 13, 14, 15):
                    emit_rope_tail(prev_st, e - 12)
                xt = xtp.tile([128, RC], BF16, tag="xt")
                nc.sync.dma_start(
                    out=xt, in_=xT[e * 128:(e + 1) * 128, col0:col0 + RC])
                stt, spp = (e == 0), (e == NE - 1)
                for h in range(HPC):
                    hsl = slice(h * D, (h + 1) * D)
                    nc.tensor.matmul(p_qk[h][0], wq_s[:, e, hsl], xt,
                                     start=stt, stop=spp)
                    nc.tensor.matmul(p_qk[h][1], wk_s[:, e, hsl], xt,
                                     start=stt, stop=spp)
                    if spp:
                        # h0's raws on DVE, h1's on ACT -- two parallel
                        # drain chains so T0 and T1 free ~concurrently
                        drain_qk(st, 2 * h + 0, use_act=(h == 1))
                        drain_qk(st, 2 * h + 1, use_act=(h == 1))
                for st4 in range(RC // 128):
                    # start=True zeroes the WHOLE psum bank; st pairs share a
                    # bank, so only the even st of each pair may set it (the
                    # odd st accumulates onto the bank-zeroed region)
                    nc.tensor.matmul((T2a, T2b)[st4 // 2][:, st4 % 2],
                                     xt[:, st4 * 128:(st4 + 1) * 128],
                                     wv_s[:, e, :],
                                     start=(stt and st4 % 2 == 0), stop=spp,
                                     skip_group_check=True)
                if spp:
                    drain_v(st)
                    for pair in range(4):
                        emit_sq(st, pair)
                if defer and e < len(defer):
                    defer[e]()
            return st

        def flush_rope(prev_st):
            """Emit the lagged rope work for the final chunk of a batch."""
            for pair in range(4):
                emit_rope_mms(prev_st, pair)
            for pair in range(4):
                emit_rope_tail(prev_st, pair)

        # ---- phase B+C per batch ---------------------------------------
        def emit_proj_unit(b, rt, chv):
            """One 512-col chunk of the output projection for row-tile rt."""
            rsl = slice(rt * 128, (rt + 1) * 128)
            p_o = (T3a, T3b)[chv % 2]
            for h in range(HPC):
                nc.tensor.matmul(
                    p_o, ctxTs[h][:, rsl],
                    wp_s[:, h, chv * 512:(chv + 1) * 512],
                    start=(h == 0), stop=(h == HPC - 1))
            o_sb = outp.tile([128, 512], BF16, tag="osb")
            if chv in (0, 2):
                nc.vector.tensor_copy(o_sb, p_o)
            else:
                nc.scalar.activation(o_sb, p_o, AF.Copy)
            nc.sync.dma_start(
                out=out[b * s_ + rt * 128: b * s_ + (rt + 1) * 128,
                        chv * 512:(chv + 1) * 512],
                in_=o_sb)

        def emit_attention(b, defer_last=False):
            for qb in range(NQB):
                qsl = slice(qb * QB, (qb + 1) * QB)
                for h in range(HPC):
                    t_ctx, t_rs = (T2a, T2b) if h == 0 else (T2b, T2a)
                    p_ctx = t_ctx.rearrange("p a b d -> p (a b d)")
                    p_rs = t_rs.rearrange("p a b d -> p (a b d)")
                    n_kt = (qb + 1) * KPQ
                    groups = []
                    for g in range(max(1, n_kt // 2)):
                        kts = [kt for kt in (2 * g, 2 * g + 1) if kt < n_kt]
                        groups.append(kts)

                    pend = []  # (kts, ex) awaiting ctx/rs matmuls

                    def flush_pend():
                        kts0, ex0 = pend.pop(0)
                        # columns [0:rel*128] of a diagonal k-tile are fully
                        # masked: skip them in the matmuls instead of zeroing
                        # (bf16 runs 1 cycle/row at any width)
                        for i, kt in enumerate(kts0):
                            c0 = max(0, kt - qb * KPQ) * 128
                            nc.tensor.matmul(p_ctx[:, c0:], vsb[h][:, kt, :],
                                             ex0[:, i * QB + c0:(i + 1) * QB],
                                             start=(kt == 0), stop=(kt == n_kt - 1),
                                             skip_group_check=True)
                        for i, kt in enumerate(kts0):
                            c0 = max(0, kt - qb * KPQ) * 128
                            nc.tensor.matmul(p_rs[:, c0:], onesb_s,
                                             ex0[:, i * QB + c0:(i + 1) * QB],
                                             start=(kt == 0), stop=(kt == n_kt - 1),
                                             skip_group_check=True)

                    for g, kts in enumerate(groups):
                        p_s = (T0, T1)[g % 2]
                        for i, kt in enumerate(kts):
                            c0 = max(0, kt - qb * KPQ) * 128
                            nc.tensor.matmul(
                                p_s[:, i, c0:],
                                ktn[h][:, kt * 128:(kt + 1) * 128],
                                qtn[h][:, qb * QB + c0:(qb + 1) * QB],
                                start=True, stop=True)
                        ex = expp.tile([128, 2 * QB], BF16, tag="ex", bufs=3)
                        rels = [kt - qb * KPQ for kt in kts]
                        if all(r < 0 for r in rels):
                            nc.scalar.activation(ex[:, :len(kts) * QB],
                                                 p_s[:, :len(kts), :], AF.Exp)
                        else:
                            for i, kt in enumerate(kts):
                                rel = rels[i]
                                esl = ex[:, i * QB:(i + 1) * QB]
                                psl = p_s[:, i]
                                if rel < 0:
                                    nc.scalar.activation(esl, psl, AF.Exp)
                                    continue
                                nc.scalar.activation(
                                    esl[:, rel * 128:], psl[:, rel * 128:], AF.Exp)
                                nc.vector.tensor_mul(
                                    esl[:, rel * 128:(rel + 1) * 128],
                                    esl[:, rel * 128:(rel + 1) * 128], tri_s)
                        pend.append((kts, ex))
                        if g >= 2:
                            flush_pend()
                    while pend:
                        flush_pend()

                    # softmax reciprocal on DVE (fast Newton approx, ~18 bits)
                    rs = tmp.tile([128, QB], F32, tag="rs", name="rs", bufs=2)
                    nc.vector.reciprocal_approx_fast(out=rs, in_=p_rs)
                    nc.vector.tensor_mul(ctxTs[h][:, qsl], p_ctx, rs)

                # output projection for this q-block's row tiles; the
                # out-DMA overlaps the next q-block's attention compute.
                # The last q-block's units are deferred into the next
                # batch's first chunk (nothing left here to overlap them).
                units = [(rt, chv)
                         for rt in range(qb * QB // 128, (qb + 1) * QB // 128)
                         for chv in range(4)]
                if defer_last and qb == NQB - 1:
                    return [(lambda b=b, rt=rt, chv=chv:
                             emit_proj_unit(b, rt, chv)) for rt, chv in units]
                for rt, chv in units:
                    emit_proj_unit(b, rt, chv)
            return None

        # ---- main loop --------------------------------------------------
        deferred = None
        for b in range(b_):
            st = None
            for rc in range(NRC):
                st = emit_chunk(b * NRC + rc, st,
                                defer=deferred if rc == 0 else None)
                if rc == 0:
                    deferred = None
            flush_rope(st)
            if debug and b == 0:
                for h in range(HPC):
                    nc.sync.dma_start(out=dbg_qtn[h], in_=qtn[h])
                    nc.sync.dma_start(out=dbg_ktn[h], in_=ktn[h])
                    nc.sync.dma_start(out=dbg_vsb[h], in_=vsb[h])
            deferred = emit_attention(b, defer_last=(b < b_ - 1))
            if debug and b == 0:
                for h in range(HPC):
                    nc.sync.dma_start(out=dbg_ctx[h], in_=ctxTs[h])

    nc.compile()
    return nc


def host_inputs(x, w_qkv, w_proj, core, s_=None):
    """Per-core input map (numpy)."""
    b_, s_x, e = x.shape
    s_ = s_x if s_ is None else s_
    xT = np.ascontiguousarray(x.reshape(b_ * s_, e).T.astype(NPBF16))

    hs = [core * HPC + i for i in range(HPC)]
    perm = np.concatenate([np.arange(0, D, 2), np.arange(1, D, 2)])
    wq_c = np.concatenate(
        [w_qkv[:, 0 * e + h * D:0 * e + (h + 1) * D][:, perm] for h in hs], axis=1)
    wk_c = np.concatenate(
        [w_qkv[:, 1 * e + h * D:1 * e + (h + 1) * D][:, perm] for h in hs], axis=1)
    wv_c = np.concatenate(
        [w_qkv[:, 2 * e + h * D:2 * e + (h + 1) * D] for h in hs], axis=1)
    wp_c = np.concatenate([w_proj[h * D:(h + 1) * D, :] for h in hs], axis=0)

    inv_freq = 1.0 / (ROPE_BASE ** (np.arange(0, D, 2, dtype=np.float64) / D))
    t = np.arange(s_, dtype=np.float64)
    freqs = np.outer(inv_freq, t)            # [64, S]
    cosT = np.cos(freqs).astype(np.float32)
    sinT = np.sin(freqs).astype(np.float32)
    cos2 = np.vstack([cosT, cosT])
    sin2 = np.vstack([sinT, sinT])

    J = np.zeros((128, 128), np.float32)
    for r in range(64):
        J[r, r + 64] = -1.0
        J[r + 64, r] = 1.0
    jmat = np.ascontiguousarray(J.T)

    ki, qi = np.meshgrid(np.arange(128), np.arange(128), indexing="ij")
    trimask = (ki <= qi).astype(NPBF16)

    return {
        "xT": xT,
        "wq": np.ascontiguousarray(wq_c.astype(NPBF16)),
        "wk": np.ascontiguousarray(wk_c.astype(NPBF16)),
        "wv": np.ascontiguousarray(wv_c.astype(NPBF16)),
        "wp": np.ascontiguousarray(wp_c.astype(NPBF16)),
        "cos2": cos2, "sin2": sin2,
        "jmat": jmat, "trimask": trimask,
        "onesd": np.ones((128, 128), np.float32),
        "onesb": np.ones((128, 128), NPBF16),
    }


_CACHE = {}


def _get_nc(b_, s_):
    key = (b_, s_)
    if key not in _CACHE:
        _CACHE[key] = build_kernel(b_, s_)
    return _CACHE[key]


def kernel(x, w_qkv, w_proj):
    x = np.asarray(x, dtype=np.float32)
    w_qkv = np.asarray(w_qkv, dtype=np.float32)
    w_proj = np.asarray(w_proj, dtype=np.float32)
    b_, s_, e = x.shape

    nc = _get_nc(b_, s_)
    in_maps = [host_inputs(x, w_qkv, w_proj, c) for c in range(N_CORES)]
    res = run_bass_kernel_spmd(nc, in_maps, list(range(N_CORES)))
    acc = res.results[0]["out"].astype(np.float32)
    for c in range(1, N_CORES):
        acc = acc + res.results[c]["out"].astype(np.float32)
    return acc.reshape(b_, s_, e)


# revision 37
# speedup vs baseline: 1.1295x; 1.1295x over previous
"""Causal self-attention (RoPE + QK-RMSNorm) Trainium2 kernel, 8-core tensor-parallel.

Problem: B=4, S=2048, E=2048, H=16 heads, D=128, fp32 in/out.
Sharding: tensor-parallel over heads -- each core computes 2 heads end-to-end
(QKV projection, RoPE, QK-norm, causal attention, output projection) and
returns a partial output [B*S, E] in bf16; the host sums the 8 partials in
fp32.

All matmul operands are bf16 (same 1 cycle/row PE rate as f32r, half the
SBUF/DMA): end-to-end numeric error ~5e-3 vs the 2e-2 gate (validated in
numpy with ml_dtypes).

Schedule (the point of this rewrite -- the PE has a 4-deep wait queue and
executes in order, so any matmul emitted while its producer is still running
parks and then BLOCKS later, ready matmuls):
  Phase A (QKV+rope+norm), per 512-column chunk c:
    - 16 e-rounds of QKV matmuls; V is produced in natural [s,d] layout
      directly via matmul(stationary=xt_tile, moving=wv) -- no PE transposes.
    - chunk c-1's rope matmuls (sum-sq ones-matmul, J rotation matmul) are
      interleaved between chunk c's e-rounds (their ACT/DVE inputs completed
      during the previous chunk) and drained to SBUF immediately by DVE.
    - chunk c-1's elementwise rope tail runs on ACT/DVE/Pool underneath
      chunk c's matmuls.
  Phase B (attention), per (q-block, head): scores into a double-buffered
  PSUM tile; exp on ACT; the ctx/rowsum matmuls are emitted one k-group
  BEHIND the scores so they never park on the ACT exp. The softmax
  reciprocal is a fast DVE Newton iteration (no ACT round trip).
  Phase C (out-proj) interleaves per q-block as before, in 512-col chunks.

PSUM = 4 mega-tiles x 2 banks, manually sliced per phase:
  T0/T1 [128,2,512]: A: q|k psums per head; B: scores double-buffer.
  T2 [128,2,4,128]:  A: V-natural psum (h,st,d); B: ctx | rowsum.
  T3 [128,2,512]:    A: rope scratch x2;        B/C: proj out double-buffer.
"""

import sys

sys.path.insert(0, "/opt/trn_rl_repo")

import numpy as np
import ml_dtypes
from contextlib import ExitStack

import concourse.bass as bass
import concourse.mybir as mybir
import concourse.tile as tile
from concourse import bacc
from concourse.bass_utils import run_bass_kernel_spmd

# Keep every ACT function this kernel uses (Exp/Ln/Square/Copy) resolvable
# only via the combined natural_log_exp_and_others table set; otherwise the
# table-load inserter alternates between exp_and_others and natural_log and
# pays a ~2.7us ACT_TABLE_LOAD on nearly every activation (~500us/run).
_orig_get_act_tables = bacc.get_activation_tables
_COMBINED = "natural_log_exp_and_others"
_KEEP = {mybir.ActivationFunctionType.Exp, mybir.ActivationFunctionType.Ln,
         mybir.ActivationFunctionType.Square, mybir.ActivationFunctionType.Copy}


def _patched_get_act_tables(arch):
    tables = _orig_get_act_tables(arch)
    if _COMBINED in tables and _KEEP <= tables[_COMBINED]:
        for name, funcs in tables.items():
            if name != _COMBINED:
                tables[name] = funcs - _KEEP
    return tables


bacc.get_activation_tables = _patched_get_act_tables

F32 = mybir.dt.float32
F32R = mybir.dt.float32r
BF16 = mybir.dt.bfloat16
AF = mybir.ActivationFunctionType
NPBF16 = ml_dtypes.bfloat16

N_CORES = 8
N_HEAD = 16
ROPE_BASE = 10000.0
QK_NORM_EPS = 1e-5

B, S, E = 4, 2048, 2048
D = E // N_HEAD          # 128
HPC = N_HEAD // N_CORES  # heads per core


def build_kernel(b_=B, s_=S, repeat=1, debug=False):
    """Build the per-core Bass program for batch size b_ and seqlen s_."""
    ROWS = b_ * s_
    QB = min(512, s_)     # q-block width in attention
    NQB = s_ // QB
    KPQ = QB // 128       # k-tiles spanned by one q-block (diag band width)
    NE = E // 128
    RC = min(512, s_)     # phase-A column chunk
    NRC = s_ // RC        # chunks per batch
    NCH = b_ * NRC        # chunks per repeat iteration

    nc = bacc.Bacc("TRN2", target_bir_lowering=False, debug=False)

    xT = nc.dram_tensor("xT", [E, ROWS], BF16, kind="ExternalInput").ap()
    wq = nc.dram_tensor("wq", [E, HPC * D], BF16, kind="ExternalInput").ap()
    wk = nc.dram_tensor("wk", [E, HPC * D], BF16, kind="ExternalInput").ap()
    wv = nc.dram_tensor("wv", [E, HPC * D], BF16, kind="ExternalInput").ap()
    wp = nc.dram_tensor("wp", [HPC * D, E], BF16, kind="ExternalInput").ap()
    cos2 = nc.dram_tensor("cos2", [128, s_], F32, kind="ExternalInput").ap()
    sin2 = nc.dram_tensor("sin2", [128, s_], F32, kind="ExternalInput").ap()
    jmat = nc.dram_tensor("jmat", [128, 128], F32R, kind="ExternalInput").ap()
    trimask = nc.dram_tensor("trimask", [128, 128], BF16, kind="ExternalInput").ap()
    onesd = nc.dram_tensor("onesd", [128, 128], F32R, kind="ExternalInput").ap()
    onesb = nc.dram_tensor("onesb", [128, 128], BF16, kind="ExternalInput").ap()
    out = nc.dram_tensor("out", [ROWS, E], BF16, kind="ExternalOutput").ap()
    if debug:
        dbg_qtn = nc.dram_tensor("dbg_qtn", [HPC, 128, s_], BF16, kind="ExternalOutput").ap()
        dbg_ktn = nc.dram_tensor("dbg_ktn", [HPC, 128, s_], BF16, kind="ExternalOutput").ap()
        dbg_vsb = nc.dram_tensor("dbg_vsb", [HPC, 128, s_ // 128, D], BF16, kind="ExternalOutput").ap()
        dbg_ctx = nc.dram_tensor("dbg_ctx", [HPC, 128, s_], BF16, kind="ExternalOutput").ap()

    LN_SCALE = 1.0 / D
    LN_BIAS = QK_NORM_EPS
    EXP_SCALE = -0.5
    EXP_BIAS = -0.25 * float(np.log(D))  # folds 1/sqrt(D) into the q,k scales

    with tile.TileContext(nc) as tc, ExitStack() as ctx:
        wpool = ctx.enter_context(tc.tile_pool(name="weights", bufs=1))
        const = ctx.enter_context(tc.tile_pool(name="const", bufs=1))
        xtp = ctx.enter_context(tc.tile_pool(name="xt", bufs=6))
        qkv = ctx.enter_context(tc.tile_pool(name="qkv", bufs=1))
        tmp = ctx.enter_context(tc.tile_pool(name="tmp", bufs=2))
        expp = ctx.enter_context(tc.tile_pool(name="expp", bufs=2))
        ctxp = ctx.enter_context(tc.tile_pool(name="ctxp", bufs=1))
        outp = ctx.enter_context(tc.tile_pool(name="outp", bufs=6))
        psp = ctx.enter_context(tc.tile_pool(name="ps", bufs=1, space="PSUM"))

        # resident weights: [128, NE, HPC*D] with contraction slice e on free dim
        wq_s = wpool.tile([128, NE, HPC * D], BF16, tag="wqs")
        wk_s = wpool.tile([128, NE, HPC * D], BF16, tag="wks")
        wv_s = wpool.tile([128, NE, HPC * D], BF16, tag="wvs")
        wp_s = wpool.tile([128, HPC, E], BF16, tag="wps")
        nc.sync.dma_start(out=wq_s, in_=wq.rearrange("(ne p) m -> p ne m", p=128))
        nc.sync.dma_start(out=wk_s, in_=wk.rearrange("(ne p) m -> p ne m", p=128))
        nc.sync.dma_start(out=wv_s, in_=wv.rearrange("(ne p) m -> p ne m", p=128))
        nc.sync.dma_start(out=wp_s, in_=wp.rearrange("(h p) m -> p h m", p=128))

        cos_s = const.tile([128, s_], F32, tag="cos")
        sin_s = const.tile([128, s_], F32, tag="sin")
        j_s = const.tile([128, 128], F32R, tag="jmat")
        tri_s = const.tile([128, 128], BF16, tag="tri")
        ones_s = const.tile([128, 128], F32R, tag="ones")
        onesb_s = const.tile([128, 128], BF16, tag="onesb")
        nc.sync.dma_start(out=ones_s, in_=onesd)
        nc.sync.dma_start(out=onesb_s, in_=onesb)
        bias_ln = const.tile([128, 1], F32, tag="bias_ln")
        bias_ex = const.tile([128, 1], F32, tag="bias_ex")
        nc.vector.memset(bias_ln, LN_BIAS)
        nc.vector.memset(bias_ex, EXP_BIAS)

        nc.sync.dma_start(out=cos_s, in_=cos2)
        nc.sync.dma_start(out=sin_s, in_=sin2)
        nc.sync.dma_start(out=j_s, in_=jmat)
        nc.sync.dma_start(out=tri_s, in_=trimask)

        # PSUM mega-tiles (2 banks each, manually sliced per phase)
        T0 = psp.tile([128, 2, 512], F32, tag="T0", name="T0")
        T1 = psp.tile([128, 2, 512], F32, tag="T1", name="T1")
        T2a = psp.tile([128, 2, 2, 128], F32, tag="T2a", name="T2a")
        T2b = psp.tile([128, 2, 2, 128], F32, tag="T2b", name="T2b")
        T3a = psp.tile([128, 512], F32, tag="T3a", name="T3a")
        T3b = psp.tile([128, 512], F32, tag="T3b", name="T3b")
        p_qk = [(T0[:, 0], T0[:, 1]), (T1[:, 0], T1[:, 1])]   # (q, k) per head

        # per-batch SBUF state (bf16)
        qtn = [qkv.tile([128, s_], BF16, tag=f"qtn{h}", name=f"qtn{h}")
               for h in range(HPC)]
        ktn = [qkv.tile([128, s_], BF16, tag=f"ktn{h}", name=f"ktn{h}")
               for h in range(HPC)]
        vsb = [qkv.tile([128, s_ // 128, D], BF16, tag=f"vsb{h}", name=f"vsb{h}")
               for h in range(HPC)]
        ctxTs = [ctxp.tile([128, s_], BF16, tag=f"ctxT{h}", name=f"ctxT{h}")
                 for h in range(HPC)]

        rep_ctx = tc.For_i(0, repeat, 1) if repeat > 1 else None
        if rep_ctx is not None:
            ctx.enter_context(rep_ctx)

        # ---- phase A helpers (software-pipelined over chunks) -----------
        # Pipeline state for the previous chunk: SBUF drains of its psums.
        prev = {}

        def drain_qk(st, pair, use_act=False):
            """psum -> SBUF raw copy of one q/k psum (the only psum reader;
            sq is computed later from the SBUF copy so it doesn't hold the
            psum tile -- tile deps are whole-tile for PSUM)."""
            h, which = divmod(pair, 2)
            psrc = p_qk[h][which]
            raw = tmp.tile([128, RC], F32R, tag="raw", name="raw", bufs=8)
            if use_act:
                nc.scalar.activation(raw, psrc, AF.Copy)
            else:
                nc.vector.tensor_copy(raw, psrc)
            st["raw"][pair] = raw

        def emit_sq(st, pair):
            sq = tmp.tile([128, RC], F32R, tag="sq", name="sq", bufs=8)
            nc.scalar.activation(sq, st["raw"][pair], AF.Square)
            st["sq"][pair] = sq

        def drain_v(st):
            c = st["c"]
            rc = c % NRC
            nst = RC // 128
            hnst = nst // 2
            for h in range(HPC):
                nc.vector.tensor_copy(
                    vsb[h][:, rc * nst:rc * nst + hnst, :], T2a[:, :, h, :])
                nc.vector.tensor_copy(
                    vsb[h][:, rc * nst + hnst:(rc + 1) * nst, :], T2b[:, :, h, :])

        def emit_rope_mms(st, pair):
            """ss/jq matmuls + immediate DVE drains for one (q/k,head) chain."""
            raw, sq = st["raw"][pair], st["sq"][pair]
            sc = (T3a, T3b)[pair % 2]
            nc.tensor.matmul(sc, ones_s, sq, start=True, stop=True)
            den = tmp.tile([128, RC], F32, tag="den", name="den", bufs=4)
            nc.vector.tensor_copy(den, sc)
            sc2 = (T3a, T3b)[(pair + 1) % 2]
            nc.tensor.matmul(sc2, j_s, raw, start=True, stop=True)
            jqr = tmp.tile([128, RC], F32, tag="jqr", name="jqr", bufs=4)
            nc.vector.tensor_copy(jqr, sc2)
            st.setdefault("den", []).append(den)
            st.setdefault("jqr", []).append(jqr)

        def emit_rope_tail(st, pair):
            """Elementwise rope+norm for one chain; writes qtn/ktn chunk c."""
            c = st["c"]
            rc = c % NRC
            csl = slice(rc * RC, rc * RC + RC)
            h, which = divmod(pair, 2)
            dst = qtn[h] if which == 0 else ktn[h]
            raw = st["raw"][pair]
            den = st["den"][pair]
            jqr = st["jqr"][pair]
            lnt = tmp.tile([128, RC], F32, tag="lnt", name="lnt", bufs=2)
            nc.scalar.activation(lnt, den, AF.Ln, scale=LN_SCALE, bias=bias_ln)
            rq = tmp.tile([128, RC], F32, tag="rq", name="rq", bufs=3)
            nc.scalar.activation(rq, lnt, AF.Exp, scale=EXP_SCALE, bias=bias_ex)
            t1 = tmp.tile([128, RC], F32, tag="t1", name="t1", bufs=2)
            nc.gpsimd.tensor_mul(t1, raw, cos_s[:, csl])
            t2 = tmp.tile([128, RC], F32, tag="t2", name="t2", bufs=2)
            nc.vector.tensor_mul(t2, jqr, sin_s[:, csl])
            t3 = tmp.tile([128, RC], F32, tag="t3", name="t3", bufs=2)
            nc.gpsimd.tensor_add(t3, t1, t2)
            nc.gpsimd.tensor_mul(dst[:, csl], t3, rq)

        def emit_chunk(c, prev_st, defer=None):
            """e-rounds for chunk c with chunk c-1's rope work interleaved.

            defer: optional list of closures (previous batch's last-qb proj
            units), one emitted per e-round to overlap with this chunk."""
            b, rc = divmod(c, NRC)
            col0 = b * s_ + rc * RC
            st = {"c": c, "raw": [None] * 4, "sq": [None] * 4}
            for e in range(NE):
                if prev_st and e in (4, 6, 8, 10):
                    emit_rope_mms(prev_st, (e - 4) // 2)
                if prev_st and e in (12, 13, 14, 15):
                    emit_rope_tail(prev_st, e - 12)
                xt = xtp.tile([128, RC], BF16, tag="xt")
                nc.sync.dma_start(
                    out=xt, in_=xT[e * 128:(e + 1) * 128, col0:col0 + RC])
                stt, spp = (e == 0), (e == NE - 1)
                for h in range(HPC):
                    hsl = slice(h * D, (h + 1) * D)
                    nc.tensor.matmul(p_qk[h][0], wq_s[:, e, hsl], xt,
                                     start=stt, stop=spp)
                    nc.tensor.matmul(p_qk[h][1], wk_s[:, e, hsl], xt,
                                     start=stt, stop=spp)
                    if spp:
                        # h0's raws on DVE, h1's on ACT -- two parallel
                        # drain chains so T0 and T1 free ~concurrently
                        drain_qk(st, 2 * h + 0, use_act=(h == 1))
                        drain_qk(st, 2 * h + 1, use_act=(h == 1))
                for st4 in range(RC // 128):
                    # start=True zeroes the WHOLE psum bank; st pairs share a
                    # bank, so only the even st of each pair may set it (the
                    # odd st accumulates onto the bank-zeroed region)
                    nc.tensor.matmul((T2a, T2b)[st4 // 2][:, st4 % 2],
                                     xt[:, st4 * 128:(st4 + 1) * 128],
                                     wv_s[:, e, :],
                                     start=(stt and st4 % 2 == 0), stop=spp,
                                     skip_group_check=True)
                if spp:
                    drain_v(st)
                    for pair in range(4):
                        emit_sq(st, pair)
                if defer and e < len(defer):
                    defer[e]()
            return st

        def flush_rope(prev_st):
            """Emit the lagged rope work for the final chunk of a batch."""
            for pair in range(4):
                emit_rope_mms(prev_st, pair)
            for pair in range(4):
                emit_rope_tail(prev_st, pair)

        # ---- phase B+C per batch ---------------------------------------
        def emit_proj_unit(b, rt, chv):
            """One 512-col chunk of the output projection for row-tile rt."""
            rsl = slice(rt * 128, (rt + 1) * 128)
            p_o = (T3a, T3b)[chv % 2]
            for h in range(HPC):
                nc.tensor.matmul(
                    p_o, ctxTs[h][:, rsl],
                    wp_s[:, h, chv * 512:(chv + 1) * 512],
                    start=(h == 0), stop=(h == HPC - 1))
            o_sb = outp.tile([128, 512], BF16, tag="osb")
            if chv in (0, 2):
                nc.vector.tensor_copy(o_sb, p_o)
            else:
                nc.scalar.activation(o_sb, p_o, AF.Copy)
            nc.sync.dma_start(
                out=out[b * s_ + rt * 128: b * s_ + (rt + 1) * 128,
                        chv * 512:(chv + 1) * 512],
                in_=o_sb)

        def emit_attention(b, defer_last=False):
            for qb in range(NQB):
                qsl = slice(qb * QB, (qb + 1) * QB)
                for h in range(HPC):
                    t_ctx, t_rs = (T2a, T2b) if h == 0 else (T2b, T2a)
                    p_ctx = t_ctx.rearrange("p a b d -> p (a b d)")
                    p_rs = t_rs.rearrange("p a b d -> p (a b d)")
                    n_kt = (qb + 1) * KPQ
                    groups = []
                    for g in range(max(1, n_kt // 2)):
                        kts = [kt for kt in (2 * g, 2 * g + 1) if kt < n_kt]
                        groups.append(kts)

                    pend = []  # (kts, ex) awaiting ctx/rs matmuls

                    def flush_pend():
                        kts0, ex0 = pend.pop(0)
                        # columns [0:rel*128] of a diagonal k-tile are fully
                        # masked: skip them in the matmuls instead of zeroing
                        # (bf16 runs 1 cycle/row at any width)
                        for i, kt in enumerate(kts0):
                            c0 = max(0, kt - qb * KPQ) * 128
                            nc.tensor.matmul(p_ctx[:, c0:], vsb[h][:, kt, :],
                                             ex0[:, i * QB + c0:(i + 1) * QB],
                                             start=(kt == 0), stop=(kt == n_kt - 1),
                                             skip_group_check=True)
                        for i, kt in enumerate(kts0):
                            c0 = max(0, kt - qb * KPQ) * 128
                            nc.tensor.matmul(p_rs[:, c0:], onesb_s,
                                             ex0[:, i * QB + c0:(i + 1) * QB],
                                             start=(kt == 0), stop=(kt == n_kt - 1),
                                             skip_group_check=True)

                    for g, kts in enumerate(groups):
                        p_s = (T0, T1)[g % 2]
                        for i, kt in enumerate(kts):
                            c0 = max(0, kt - qb * KPQ) * 128
                            nc.tensor.matmul(
                                p_s[:, i, c0:],
                                ktn[h][:, kt * 128:(kt + 1) * 128],
                                qtn[h][:, qb * QB + c0:(qb + 1) * QB],
                                start=True, stop=True)
                        ex = expp.tile([128, 2 * QB], BF16, tag="ex", bufs=3)
                        rels = [kt - qb * KPQ for kt in kts]
                        if all(r < 0 for r in rels):
                            nc.scalar.activation(ex[:, :len(kts) * QB],
                                                 p_s[:, :len(kts), :], AF.Exp)
                        else:
                            for i, kt in enumerate(kts):
                                rel = rels[i]
                                esl = ex[:, i * QB:(i + 1) * QB]
                                psl = p_s[:, i]
                                if rel < 0:
                                    nc.scalar.activation(esl, psl, AF.Exp)
                                    continue
                                nc.scalar.activation(
                                    esl[:, rel * 128:], psl[:, rel * 128:], AF.Exp)
                                nc.vector.tensor_mul(
                                    esl[:, rel * 128:(rel + 1) * 128],
                                    esl[:, rel * 128:(rel + 1) * 128], tri_s)
                        pend.append((kts, ex))
                        if g >= 2:
                            flush_pend()
                    while pend:
                        flush_pend()

                    # softmax reciprocal on DVE (fast Newton approx, ~18 bits)
                    rs = tmp.tile([128, QB], F32, tag="rs", name="rs", bufs=2)
                    nc.vector.reciprocal_approx_fast(out=rs, in_=p_rs)
                    nc.vector.tensor_mul(ctxTs[h][:, qsl], p_ctx, rs)

                # output projection for this q-block's row tiles; the
                # out-DMA overlaps the next q-block's attention compute.
                # The last q-block's units are deferred into the next
                # batch's first chunk (nothing left here to overlap them).
                units = [(rt, chv)
                         for rt in range(qb * QB // 128, (qb + 1) * QB // 128)
                         for chv in range(4)]
                if defer_last and qb == NQB - 1:
                    return [(lambda b=b, rt=rt, chv=chv:
                             emit_proj_unit(b, rt, chv)) for rt, chv in units]
                for rt, chv in units:
                    emit_proj_unit(b, rt, chv)
            return None

        # ---- main loop --------------------------------------------------
        deferred = None
        for b in range(b_):
            st = None
            for rc in range(NRC):
                st = emit_chunk(b * NRC + rc, st,
                                defer=deferred if rc == 0 else None)
                if rc == 0:
                    deferred = None
            flush_rope(st)
            if debug and b == 0:
                for h in range(HPC):
                    nc.sync.dma_start(out=dbg_qtn[h], in_=qtn[h])
                    nc.sync.dma_start(out=dbg_ktn[h], in_=ktn[h])
                    nc.sync.dma_start(out=dbg_vsb[h], in_=vsb[h])
            deferred = emit_attention(b, defer_last=(b < b_ - 1))
            if debug and b == 0:
                for h in range(HPC):
                    nc.sync.dma_start(out=dbg_ctx[h], in_=ctxTs[h])

    nc.compile()
    return nc


def host_inputs(x, w_qkv, w_proj, core, s_=None):
    """Per-core input map (numpy)."""
    b_, s_x, e = x.shape
    s_ = s_x if s_ is None else s_
    xT = np.ascontiguousarray(x.reshape(b_ * s_, e).T.astype(NPBF16))

    hs = [core * HPC + i for i in range(HPC)]
    perm = np.concatenate([np.arange(0, D, 2), np.arange(1, D, 2)])
    wq_c = np.concatenate(
        [w_qkv[:, 0 * e + h * D:0 * e + (h + 1) * D][:, perm] for h in hs], axis=1)
    wk_c = np.concatenate(
        [w_qkv[:, 1 * e + h * D:1 * e + (h + 1) * D][:, perm] for h in hs], axis=1)
    wv_c = np.concatenate(
        [w_qkv[:, 2 * e + h * D:2 * e + (h + 1) * D] for h in hs], axis=1)
    wp_c = np.concatenate([w_proj[h * D:(h + 1) * D, :] for h in hs], axis=0)

    inv_freq = 1.0 / (ROPE_BASE ** (np.arange(0, D, 2, dtype=np.float64) / D))
    t = np.arange(s_, dtype=np.float64)
    freqs = np.outer(inv_freq, t)            # [64, S]
    cosT = np.cos(freqs).astype(np.float32)
    sinT = np.sin(freqs).astype(np.float32)
    cos2 = np.vstack([cosT, cosT])
    sin2 = np.vstack([sinT, sinT])

    J = np.zeros((128, 128), np.float32)
    for r in range(64):
        J[r, r + 64] = -1.0
        J[r + 64, r] = 1.0
    jmat = np.ascontiguousarray(J.T)

    ki, qi = np.meshgrid(np.arange(128), np.arange(128), indexing="ij")
    trimask = (ki <= qi).astype(NPBF16)

    return {
        "xT": xT,
        "wq": np.ascontiguousarray(wq_c.astype(NPBF16)),
        "wk": np.ascontiguousarray(wk_c.astype(NPBF16)),
        "wv": np.ascontiguousarray(wv_c.astype(NPBF16)),
        "wp": np.ascontiguousarray(wp_c.astype(NPBF16)),
        "cos2": cos2, "sin2": sin2,
        "jmat": jmat, "trimask": trimask,
        "onesd": np.ones((128, 128), np.float32),
        "onesb": np.ones((128, 128), NPBF16),
    }


_CACHE = {}


def _get_nc(b_, s_):
    key = (b_, s_)
    if key not in _CACHE:
        _CACHE[key] = build_kernel(b_, s_)
    return _CACHE[key]


def kernel(x, w_qkv, w_proj):
    x = np.asarray(x, dtype=np.float32)
    w_qkv = np.asarray(w_qkv, dtype=np.float32)
    w_proj = np.asarray(w_proj, dtype=np.float32)
    b_, s_, e = x.shape

    nc = _get_nc(b_, s_)
    in_maps = [host_inputs(x, w_qkv, w_proj, c) for c in range(N_CORES)]
    res = run_bass_kernel_spmd(nc, in_maps, list(range(N_CORES)))
    acc = res.results[0]["out"].astype(np.float32)
    for c in range(1, N_CORES):
        acc = acc + res.results[c]["out"].astype(np.float32)
    return acc.reshape(b_, s_, e)


# revision 40
# speedup vs baseline: 1.1471x; 1.0156x over previous
"""Causal self-attention (RoPE + QK-RMSNorm) Trainium2 kernel, 8-core tensor-parallel.

Problem: B=4, S=2048, E=2048, H=16 heads, D=128, fp32 in/out.
Sharding: tensor-parallel over heads -- each core computes 2 heads end-to-end
(QKV projection, RoPE, QK-norm, causal attention, output projection) and
returns a partial output [B*S, E] in bf16; the host sums the 8 partials in
fp32.

All matmul operands are bf16 (same 1 cycle/row PE rate as f32r, half the
SBUF/DMA): end-to-end numeric error ~5e-3 vs the 2e-2 gate (validated in
numpy with ml_dtypes).

Schedule (the point of this rewrite -- the PE has a 4-deep wait queue and
executes in order, so any matmul emitted while its producer is still running
parks and then BLOCKS later, ready matmuls):
  Phase A (QKV+rope+norm), per 512-column chunk c:
    - 16 e-rounds of QKV matmuls; V is produced in natural [s,d] layout
      directly via matmul(stationary=xt_tile, moving=wv) -- no PE transposes.
    - chunk c-1's rope matmuls (sum-sq ones-matmul, J rotation matmul) are
      interleaved between chunk c's e-rounds (their ACT/DVE inputs completed
      during the previous chunk) and drained to SBUF immediately by DVE.
    - chunk c-1's elementwise rope tail runs on ACT/DVE/Pool underneath
      chunk c's matmuls.
  Phase B (attention), per (q-block, head): scores into a double-buffered
  PSUM tile; exp on ACT; the ctx/rowsum matmuls are emitted one k-group
  BEHIND the scores so they never park on the ACT exp. The softmax
  reciprocal is a fast DVE Newton iteration (no ACT round trip).
  Phase C (out-proj) interleaves per q-block as before, in 512-col chunks.

PSUM = 8 banks, manually assigned per phase (psum tile deps are whole-tile,
so independently-consumed psums get separate tiles):
  T0/T1 [128,2,512] (2 banks each): A: q|k psums per head; B: scores 2-buf.
  T2a/T2b [128,2,2,128] (1 bank each): A: V-natural psum (st-pair, h, d)
    -- note matmul start=True zeroes the WHOLE bank, so only the first
    st of each pair sets it; B: ctx | rowsum (swapped between heads so a
    head's first ctx matmul waits only the other head's cheap recip).
  T3a/T3b [128,512] (1 bank each): A: rope scratch; B/C: proj out 2-buf.
"""

import sys

sys.path.insert(0, "/opt/trn_rl_repo")

import numpy as np
import ml_dtypes
from contextlib import ExitStack

import concourse.bass as bass
import concourse.mybir as mybir
import concourse.tile as tile
from concourse import bacc
from concourse.bass_utils import run_bass_kernel_spmd

# Keep every ACT function this kernel uses (Exp/Ln/Square/Copy) resolvable
# only via the combined natural_log_exp_and_others table set; otherwise the
# table-load inserter alternates between exp_and_others and natural_log and
# pays a ~2.7us ACT_TABLE_LOAD on nearly every activation (~500us/run).
_orig_get_act_tables = bacc.get_activation_tables
_COMBINED = "natural_log_exp_and_others"
_KEEP = {mybir.ActivationFunctionType.Exp, mybir.ActivationFunctionType.Ln,
         mybir.ActivationFunctionType.Square, mybir.ActivationFunctionType.Copy}


def _patched_get_act_tables(arch):
    tables = _orig_get_act_tables(arch)
    if _COMBINED in tables and _KEEP <= tables[_COMBINED]:
        for name, funcs in tables.items():
            if name != _COMBINED:
                tables[name] = funcs - _KEEP
    return tables


bacc.get_activation_tables = _patched_get_act_tables

F32 = mybir.dt.float32
F32R = mybir.dt.float32r
BF16 = mybir.dt.bfloat16
AF = mybir.ActivationFunctionType
NPBF16 = ml_dtypes.bfloat16

N_CORES = 8
N_HEAD = 16
ROPE_BASE = 10000.0
QK_NORM_EPS = 1e-5

B, S, E = 4, 2048, 2048
D = E // N_HEAD          # 128
HPC = N_HEAD // N_CORES  # heads per core


def build_kernel(b_=B, s_=S, repeat=1, debug=False):
    """Build the per-core Bass program for batch size b_ and seqlen s_."""
    ROWS = b_ * s_
    QB = min(512, s_)     # q-block width in attention
    NQB = s_ // QB
    KPQ = QB // 128       # k-tiles spanned by one q-block (diag band width)
    NE = E // 128
    RC = min(512, s_)     # phase-A column chunk
    NRC = s_ // RC        # chunks per batch
    NCH = b_ * NRC        # chunks per repeat iteration

    nc = bacc.Bacc("TRN2", target_bir_lowering=False, debug=False)

    xT = nc.dram_tensor("xT", [E, ROWS], BF16, kind="ExternalInput").ap()
    wq = nc.dram_tensor("wq", [E, HPC * D], BF16, kind="ExternalInput").ap()
    wk = nc.dram_tensor("wk", [E, HPC * D], BF16, kind="ExternalInput").ap()
    wv = nc.dram_tensor("wv", [E, HPC * D], BF16, kind="ExternalInput").ap()
    wp = nc.dram_tensor("wp", [HPC * D, E], BF16, kind="ExternalInput").ap()
    cos2 = nc.dram_tensor("cos2", [128, s_], F32, kind="ExternalInput").ap()
    sin2 = nc.dram_tensor("sin2", [128, s_], F32, kind="ExternalInput").ap()
    jmat = nc.dram_tensor("jmat", [128, 128], F32R, kind="ExternalInput").ap()
    trimask = nc.dram_tensor("trimask", [128, 128], BF16, kind="ExternalInput").ap()
    onesd = nc.dram_tensor("onesd", [128, 128], F32R, kind="ExternalInput").ap()
    onesb = nc.dram_tensor("onesb", [128, 128], BF16, kind="ExternalInput").ap()
    out = nc.dram_tensor("out", [ROWS, E], BF16, kind="ExternalOutput").ap()
    if debug:
        dbg_qtn = nc.dram_tensor("dbg_qtn", [HPC, 128, s_], BF16, kind="ExternalOutput").ap()
        dbg_ktn = nc.dram_tensor("dbg_ktn", [HPC, 128, s_], BF16, kind="ExternalOutput").ap()
        dbg_vsb = nc.dram_tensor("dbg_vsb", [HPC, 128, s_ // 128, D], BF16, kind="ExternalOutput").ap()
        dbg_ctx = nc.dram_tensor("dbg_ctx", [HPC, 128, s_], BF16, kind="ExternalOutput").ap()

    LN_SCALE = 1.0 / D
    LN_BIAS = QK_NORM_EPS
    EXP_SCALE = -0.5
    EXP_BIAS = -0.25 * float(np.log(D))  # folds 1/sqrt(D) into the q,k scales

    with tile.TileContext(nc) as tc, ExitStack() as ctx:
        wpool = ctx.enter_context(tc.tile_pool(name="weights", bufs=1))
        const = ctx.enter_context(tc.tile_pool(name="const", bufs=1))
        xtp = ctx.enter_context(tc.tile_pool(name="xt", bufs=6))
        qkv = ctx.enter_context(tc.tile_pool(name="qkv", bufs=1))
        tmp = ctx.enter_context(tc.tile_pool(name="tmp", bufs=2))
        expp = ctx.enter_context(tc.tile_pool(name="expp", bufs=2))
        ctxp = ctx.enter_context(tc.tile_pool(name="ctxp", bufs=1))
        outp = ctx.enter_context(tc.tile_pool(name="outp", bufs=6))
        psp = ctx.enter_context(tc.tile_pool(name="ps", bufs=1, space="PSUM"))

        # resident weights: [128, NE, HPC*D] with contraction slice e on free dim
        wq_s = wpool.tile([128, NE, HPC * D], BF16, tag="wqs")
        wk_s = wpool.tile([128, NE, HPC * D], BF16, tag="wks")
        wv_s = wpool.tile([128, NE, HPC * D], BF16, tag="wvs")
        wp_s = wpool.tile([128, HPC, E], BF16, tag="wps")
        nc.sync.dma_start(out=wq_s, in_=wq.rearrange("(ne p) m -> p ne m", p=128))
        nc.sync.dma_start(out=wk_s, in_=wk.rearrange("(ne p) m -> p ne m", p=128))
        nc.sync.dma_start(out=wv_s, in_=wv.rearrange("(ne p) m -> p ne m", p=128))
        nc.sync.dma_start(out=wp_s, in_=wp.rearrange("(h p) m -> p h m", p=128))

        cos_s = const.tile([128, s_], F32, tag="cos")
        sin_s = const.tile([128, s_], F32, tag="sin")
        j_s = const.tile([128, 128], F32R, tag="jmat")
        tri_s = const.tile([128, 128], BF16, tag="tri")
        ones_s = const.tile([128, 128], F32R, tag="ones")
        onesb_s = const.tile([128, 128], BF16, tag="onesb")
        nc.sync.dma_start(out=ones_s, in_=onesd)
        nc.sync.dma_start(out=onesb_s, in_=onesb)
        bias_ln = const.tile([128, 1], F32, tag="bias_ln")
        bias_ex = const.tile([128, 1], F32, tag="bias_ex")
        nc.vector.memset(bias_ln, LN_BIAS)
        nc.vector.memset(bias_ex, EXP_BIAS)

        nc.sync.dma_start(out=cos_s, in_=cos2)
        nc.sync.dma_start(out=sin_s, in_=sin2)
        nc.sync.dma_start(out=j_s, in_=jmat)
        nc.sync.dma_start(out=tri_s, in_=trimask)

        # PSUM mega-tiles (2 banks each, manually sliced per phase)
        T0 = psp.tile([128, 2, 512], F32, tag="T0", name="T0")
        T1 = psp.tile([128, 2, 512], F32, tag="T1", name="T1")
        T2a = psp.tile([128, 2, 2, 128], F32, tag="T2a", name="T2a")
        T2b = psp.tile([128, 2, 2, 128], F32, tag="T2b", name="T2b")
        T3a = psp.tile([128, 512], F32, tag="T3a", name="T3a")
        T3b = psp.tile([128, 512], F32, tag="T3b", name="T3b")
        p_qk = [(T0[:, 0], T0[:, 1]), (T1[:, 0], T1[:, 1])]   # (q, k) per head

        # per-batch SBUF state (bf16)
        qtn = [qkv.tile([128, s_], BF16, tag=f"qtn{h}", name=f"qtn{h}")
               for h in range(HPC)]
        ktn = [qkv.tile([128, s_], BF16, tag=f"ktn{h}", name=f"ktn{h}")
               for h in range(HPC)]
        vsb = [qkv.tile([128, s_ // 128, D], BF16, tag=f"vsb{h}", name=f"vsb{h}")
               for h in range(HPC)]
        ctxTs = [ctxp.tile([128, s_], BF16, tag=f"ctxT{h}", name=f"ctxT{h}")
                 for h in range(HPC)]

        rep_ctx = tc.For_i(0, repeat, 1) if repeat > 1 else None
        if rep_ctx is not None:
            ctx.enter_context(rep_ctx)

        # ---- phase A helpers (software-pipelined over chunks) -----------
        # Pipeline state for the previous chunk: SBUF drains of its psums.
        prev = {}

        def drain_qk(st, pair, use_act=False):
            """psum -> SBUF raw copy of one q/k psum (the only psum reader;
            sq is computed later from the SBUF copy so it doesn't hold the
            psum tile -- tile deps are whole-tile for PSUM)."""
            h, which = divmod(pair, 2)
            psrc = p_qk[h][which]
            raw = tmp.tile([128, RC], F32R, tag="raw", name="raw", bufs=8)
            if use_act:
                nc.scalar.activation(raw, psrc, AF.Copy)
            else:
                nc.vector.tensor_copy(raw, psrc)
            st["raw"][pair] = raw

        def emit_sq(st, pair):
            sq = tmp.tile([128, RC], F32R, tag="sq", name="sq", bufs=8)
            nc.scalar.activation(sq, st["raw"][pair], AF.Square)
            st["sq"][pair] = sq

        def drain_v(st):
            c = st["c"]
            rc = c % NRC
            nst = RC // 128
            hnst = nst // 2
            for h in range(HPC):
                nc.vector.tensor_copy(
                    vsb[h][:, rc * nst:rc * nst + hnst, :], T2a[:, :, h, :])
                nc.vector.tensor_copy(
                    vsb[h][:, rc * nst + hnst:(rc + 1) * nst, :], T2b[:, :, h, :])

        def emit_rope_mms(st, pair):
            """ss/jq matmuls + immediate DVE drains for one (q/k,head) chain."""
            raw, sq = st["raw"][pair], st["sq"][pair]
            sc = (T3a, T3b)[pair % 2]
            nc.tensor.matmul(sc, ones_s, sq, start=True, stop=True)
            den = tmp.tile([128, RC], F32, tag="den", name="den", bufs=4)
            nc.vector.tensor_copy(den, sc)
            sc2 = (T3a, T3b)[(pair + 1) % 2]
            nc.tensor.matmul(sc2, j_s, raw, start=True, stop=True)
            jqr = tmp.tile([128, RC], F32, tag="jqr", name="jqr", bufs=4)
            nc.vector.tensor_copy(jqr, sc2)
            st.setdefault("den", []).append(den)
            st.setdefault("jqr", []).append(jqr)

        def emit_rope_tail(st, pair):
            """Elementwise rope+norm for one chain; writes qtn/ktn chunk c."""
            c = st["c"]
            rc = c % NRC
            csl = slice(rc * RC, rc * RC + RC)
            h, which = divmod(pair, 2)
            dst = qtn[h] if which == 0 else ktn[h]
            raw = st["raw"][pair]
            den = st["den"][pair]
            jqr = st["jqr"][pair]
            lnt = tmp.tile([128, RC], F32, tag="lnt", name="lnt", bufs=2)
            nc.scalar.activation(lnt, den, AF.Ln, scale=LN_SCALE, bias=bias_ln)
            rq = tmp.tile([128, RC], F32, tag="rq", name="rq", bufs=3)
            nc.scalar.activation(rq, lnt, AF.Exp, scale=EXP_SCALE, bias=bias_ex)
            t1 = tmp.tile([128, RC], F32, tag="t1", name="t1", bufs=2)
            nc.gpsimd.tensor_mul(t1, raw, cos_s[:, csl])
            t2 = tmp.tile([128, RC], F32, tag="t2", name="t2", bufs=2)
            nc.vector.tensor_mul(t2, jqr, sin_s[:, csl])
            t3 = tmp.tile([128, RC], F32, tag="t3", name="t3", bufs=2)
            nc.gpsimd.tensor_add(t3, t1, t2)
            nc.gpsimd.tensor_mul(dst[:, csl], t3, rq)

        def emit_chunk(c, prev_st, defer=None):
            """e-rounds for chunk c with chunk c-1's rope work interleaved.

            defer: optional list of closures (previous batch's last-qb proj
            units), one emitted per e-round to overlap with this chunk."""
            b, rc = divmod(c, NRC)
            col0 = b * s_ + rc * RC
            st = {"c": c, "raw": [None] * 4, "sq": [None] * 4}
            for e in range(NE):
                if prev_st and e in (4, 6, 8, 10):
                    emit_rope_mms(prev_st, (e - 4) // 2)
                if prev_st and e in (12, 13, 14, 15):
                    emit_rope_tail(prev_st, e - 12)
                xt = xtp.tile([128, RC], BF16, tag="xt")
                nc.sync.dma_start(
                    out=xt, in_=xT[e * 128:(e + 1) * 128, col0:col0 + RC])
                stt, spp = (e == 0), (e == NE - 1)
                for h in range(HPC):
                    hsl = slice(h * D, (h + 1) * D)
                    nc.tensor.matmul(p_qk[h][0], wq_s[:, e, hsl], xt,
                                     start=stt, stop=spp)
                    nc.tensor.matmul(p_qk[h][1], wk_s[:, e, hsl], xt,
                                     start=stt, stop=spp)
                    if spp:
                        # h0's raws on DVE, h1's on ACT -- two parallel
                        # drain chains so T0 and T1 free ~concurrently
                        drain_qk(st, 2 * h + 0, use_act=(h == 1))
                        drain_qk(st, 2 * h + 1, use_act=(h == 1))
                for st4 in range(RC // 128):
                    # start=True zeroes the WHOLE psum bank; st pairs share a
                    # bank, so only the even st of each pair may set it (the
                    # odd st accumulates onto the bank-zeroed region)
                    nc.tensor.matmul((T2a, T2b)[st4 // 2][:, st4 % 2],
                                     xt[:, st4 * 128:(st4 + 1) * 128],
                                     wv_s[:, e, :],
                                     start=(stt and st4 % 2 == 0), stop=spp,
                                     skip_group_check=True)
                if spp:
                    drain_v(st)
                    for pair in range(4):
                        emit_sq(st, pair)
                if defer and e < len(defer):
                    defer[e]()
            return st

        def flush_rope(prev_st):
            """Emit the lagged rope work for the final chunk of a batch."""
            for pair in range(4):
                emit_rope_mms(prev_st, pair)
            for pair in range(4):
                emit_rope_tail(prev_st, pair)

        # ---- phase B+C per batch ---------------------------------------
        def emit_proj_unit(b, rt, chv):
            """One 512-col chunk of the output projection for row-tile rt."""
            rsl = slice(rt * 128, (rt + 1) * 128)
            p_o = (T3a, T3b)[chv % 2]
            for h in range(HPC):
                nc.tensor.matmul(
                    p_o, ctxTs[h][:, rsl],
                    wp_s[:, h, chv * 512:(chv + 1) * 512],
                    start=(h == 0), stop=(h == HPC - 1))
            o_sb = outp.tile([128, 512], BF16, tag="osb")
            if chv in (0, 2):
                nc.vector.tensor_copy(o_sb, p_o)
            else:
                nc.scalar.activation(o_sb, p_o, AF.Copy)
            nc.sync.dma_start(
                out=out[b * s_ + rt * 128: b * s_ + (rt + 1) * 128,
                        chv * 512:(chv + 1) * 512],
                in_=o_sb)

        def emit_attention(b, defer_last=False):
            for qb in range(NQB):
                qsl = slice(qb * QB, (qb + 1) * QB)
                for h in range(HPC):
                    t_ctx, t_rs = (T2a, T2b) if h == 0 else (T2b, T2a)
                    p_ctx = t_ctx.rearrange("p a b d -> p (a b d)")
                    p_rs = t_rs.rearrange("p a b d -> p (a b d)")
                    n_kt = (qb + 1) * KPQ
                    groups = []
                    for g in range(max(1, n_kt // 2)):
                        kts = [kt for kt in (2 * g, 2 * g + 1) if kt < n_kt]
                        groups.append(kts)

                    pend = []  # (kts, ex) awaiting ctx/rs matmuls

                    def flush_pend():
                        kts0, ex0 = pend.pop(0)
                        # columns [0:rel*128] of a diagonal k-tile are fully
                        # masked: skip them in the matmuls instead of zeroing
                        # (bf16 runs 1 cycle/row at any width)
                        for i, kt in enumerate(kts0):
                            c0 = max(0, kt - qb * KPQ) * 128
                            nc.tensor.matmul(p_ctx[:, c0:], vsb[h][:, kt, :],
                                             ex0[:, i * QB + c0:(i + 1) * QB],
                                             start=(kt == 0), stop=(kt == n_kt - 1),
                                             skip_group_check=True)
                        for i, kt in enumerate(kts0):
                            c0 = max(0, kt - qb * KPQ) * 128
                            nc.tensor.matmul(p_rs[:, c0:], onesb_s,
                                             ex0[:, i * QB + c0:(i + 1) * QB],
                                             start=(kt == 0), stop=(kt == n_kt - 1),
                                             skip_group_check=True)

                    for g, kts in enumerate(groups):
                        p_s = (T0, T1)[g % 2]
                        for i, kt in enumerate(kts):
                            c0 = max(0, kt - qb * KPQ) * 128
                            nc.tensor.matmul(
                                p_s[:, i, c0:],
                                ktn[h][:, kt * 128:(kt + 1) * 128],
                                qtn[h][:, qb * QB + c0:(qb + 1) * QB],
                                start=True, stop=True)
                        ex = expp.tile([128, 2 * QB], BF16, tag="ex", bufs=3)
                        rels = [kt - qb * KPQ for kt in kts]
                        if all(r < 0 for r in rels):
                            nc.scalar.activation(ex[:, :len(kts) * QB],
                                                 p_s[:, :len(kts), :], AF.Exp)
                        else:
                            for i, kt in enumerate(kts):
                                rel = rels[i]
                                esl = ex[:, i * QB:(i + 1) * QB]
                                psl = p_s[:, i]
                                if rel < 0:
                                    nc.scalar.activation(esl, psl, AF.Exp)
                                    continue
                                nc.scalar.activation(
                                    esl[:, rel * 128:], psl[:, rel * 128:], AF.Exp)
                                nc.vector.tensor_mul(
                                    esl[:, rel * 128:(rel + 1) * 128],
                                    esl[:, rel * 128:(rel + 1) * 128], tri_s)
                        pend.append((kts, ex))
                        if g >= 2:
                            flush_pend()
                    while pend:
                        flush_pend()

                    # softmax reciprocal on DVE (fast Newton approx, ~18 bits)
                    rs = tmp.tile([128, QB], F32, tag="rs", name="rs", bufs=2)
                    nc.vector.reciprocal_approx_fast(out=rs, in_=p_rs)
                    nc.vector.tensor_mul(ctxTs[h][:, qsl], p_ctx, rs)

                # output projection for this q-block's row tiles; the
                # out-DMA overlaps the next q-block's attention compute.
                # The last q-block's units are deferred into the next
                # batch's first chunk (nothing left here to overlap them).
                units = [(rt, chv)
                         for rt in range(qb * QB // 128, (qb + 1) * QB // 128)
                         for chv in range(4)]
                if defer_last and qb == NQB - 1:
                    return [(lambda b=b, rt=rt, chv=chv:
                             emit_proj_unit(b, rt, chv)) for rt, chv in units]
                for rt, chv in units:
                    emit_proj_unit(b, rt, chv)
            return None

        # ---- main loop --------------------------------------------------
        deferred = None
        for b in range(b_):
            st = None
            for rc in range(NRC):
                st = emit_chunk(b * NRC + rc, st,
                                defer=deferred if rc == 0 else None)
                if rc == 0:
                    deferred = None
            flush_rope(st)
            if debug and b == 0:
                for h in range(HPC):
                    nc.sync.dma_start(out=dbg_qtn[h], in_=qtn[h])
                    nc.sync.dma_start(out=dbg_ktn[h], in_=ktn[h])
                    nc.sync.dma_start(out=dbg_vsb[h], in_=vsb[h])
            deferred = emit_attention(b, defer_last=(b < b_ - 1))
            if debug and b == 0:
                for h in range(HPC):
                    nc.sync.dma_start(out=dbg_ctx[h], in_=ctxTs[h])

    nc.compile()
    return nc


def host_inputs(x, w_qkv, w_proj, core, s_=None):
    """Per-core input map (numpy)."""
    b_, s_x, e = x.shape
    s_ = s_x if s_ is None else s_
    xT = np.ascontiguousarray(x.reshape(b_ * s_, e).T.astype(NPBF16))

    hs = [core * HPC + i for i in range(HPC)]
    perm = np.concatenate([np.arange(0, D, 2), np.arange(1, D, 2)])
    wq_c = np.concatenate(
        [w_qkv[:, 0 * e + h * D:0 * e + (h + 1) * D][:, perm] for h in hs], axis=1)
    wk_c = np.concatenate(
        [w_qkv[:, 1 * e + h * D:1 * e + (h + 1) * D][:, perm] for h in hs], axis=1)
    wv_c = np.concatenate(
        [w_qkv[:, 2 * e + h * D:2 * e + (h + 1) * D] for h in hs], axis=1)
    wp_c = np.concatenate([w_proj[h * D:(h + 1) * D, :] for h in hs], axis=0)

    inv_freq = 1.0 / (ROPE_BASE ** (np.arange(0, D, 2, dtype=np.float64) / D))
    t = np.arange(s_, dtype=np.float64)
    freqs = np.outer(inv_freq, t)            # [64, S]
    cosT = np.cos(freqs).astype(np.float32)
    sinT = np.sin(freqs).astype(np.float32)
    cos2 = np.vstack([cosT, cosT])
    sin2 = np.vstack([sinT, sinT])

    J = np.zeros((128, 128), np.float32)
    for r in range(64):
        J[r, r + 64] = -1.0
        J[r + 64, r] = 1.0
    jmat = np.ascontiguousarray(J.T)

    ki, qi = np.meshgrid(np.arange(128), np.arange(128), indexing="ij")
    trimask = (ki <= qi).astype(NPBF16)

    return {
        "xT": xT,
        "wq": np.ascontiguousarray(wq_c.astype(NPBF16)),
        "wk": np.ascontiguousarray(wk_c.astype(NPBF16)),
        "wv": np.ascontiguousarray(wv_c.astype(NPBF16)),
        "wp": np.ascontiguousarray(wp_c.astype(NPBF16)),
        "cos2": cos2, "sin2": sin2,
        "jmat": jmat, "trimask": trimask,
        "onesd": np.ones((128, 128), np.float32),
        "onesb": np.ones((128, 128), NPBF16),
    }


_CACHE = {}


def _get_nc(b_, s_):
    key = (b_, s_)
    if key not in _CACHE:
        _CACHE[key] = build_kernel(b_, s_)
    return _CACHE[key]


def kernel(x, w_qkv, w_proj):
    x = np.asarray(x, dtype=np.float32)
    w_qkv = np.asarray(w_qkv, dtype=np.float32)
    w_proj = np.asarray(w_proj, dtype=np.float32)
    b_, s_, e = x.shape

    nc = _get_nc(b_, s_)
    in_maps = [host_inputs(x, w_qkv, w_proj, c) for c in range(N_CORES)]
    res = run_bass_kernel_spmd(nc, in_maps, list(range(N_CORES)))
    acc = res.results[0]["out"].astype(np.float32)
    for c in range(1, N_CORES):
        acc = acc + res.results[c]["out"].astype(np.float32)
    return acc.reshape(b_, s_, e)
